# revision 1
# baseline (speedup 1.0000x reference)
"""Trainium2 Bass kernel v3 for the hypernet-MoE model (nn_BaseModel_53455162966557).

Math per sample b:
    h  = relu(relu(x @ W0 + b0) @ W1 + b1)                    [B, D]
    c  = relu(context @ Wh1 + bh1)                            [B, H]
    z  = relu(sum_k c_k (h @ W1_k) + h @ Bh2W1 + c @ Wh2b1 + bh2b1)
    o  = relu(sum_k c_k (z @ W2_k) + z @ Bh2W2 + c @ Wh2b2 + bh2b2 + h)

Design (from measured TRN2 op costs):
  - Stream matmuls: bf16 hT (lhsT) x fp8-e3m4 Wh2 (rhs, scaled by S=512
    host-side), N=512, two 128-contraction MMs per 2-k PSUM region.
    LDWEIGHTS overlaps with MM so no weight-reload tax.
  - Per-k scale+accumulate (z += c_k * A_k) is spread over four lanes:
      A (DVE):    stt direct from PSUM into bf16 chunk-acc accD
      B (DVE):    ACT bulk-copies the 4-k slab to SBUF bf16, DVE stt -> accD
      G (GPSIMD): ACT bulk-copy + gpsimd stt -> accG
      P (ACT+PE): ACT scale-copy to bf16 tmp, PE ident-matmul accumulate
                  into the init/ident PSUM region pz
    bf16 chunk-accs flush into an f32 master every CHUNK k's.
  - Sharding: pure data parallel, batch 2048 -> 8 cores x 256 rows.
"""

import ml_dtypes
import numpy as np

import concourse.bass as bass
import concourse.tile as tile
from concourse import bacc, mybir
from concourse.masks import make_identity

F32 = mybir.dt.float32
BF16 = mybir.dt.bfloat16
FP8E3 = mybir.dt.float8e3
AF = mybir.ActivationFunctionType
ALU = mybir.AluOpType

B, OBS, CTX, D, M, H = 2048, 64, 16, 256, 256, 256
NCORES = 8
BS = B // NCORES
I0 = D * M
I1 = I0 + M
I2 = I1 + M * D
NPARAMS = I2 + D
S = 512.0
G = 16      # k's per DMA group
NGROUPS = H // G
SLAB = 4    # k's per psum slab
NSLABS = G // SLAB
CHUNK = 32  # k's per bf16 chunk-acc before f32 flush


def _make_schedule():
    """64 slab-lane assignments per (layer, half): B=21, A=13, G=21, P=9."""
    counts = {'A': 31, 'G': 16, 'P': 17}
    total = sum(counts.values())
    sched = []
    err = {k: 0.0 for k in counts}
    for i in range(total):
        for k in counts:
            err[k] += counts[k] / total
        pick = max(err, key=lambda k: err[k])
        err[pick] -= 1.0
        sched.append(pick)
    return sched


SCHEDULE = _make_schedule()

_CACHED_NC = None


def build_nc():
    nc = bacc.Bacc("TRN2", target_bir_lowering=False, debug=False)

    x = nc.dram_tensor("x", [BS, OBS], F32, kind="ExternalInput")
    ctx_in = nc.dram_tensor("context", [BS, CTX], F32, kind="ExternalInput")
    W0 = nc.dram_tensor("W0", [OBS, D], F32, kind="ExternalInput")
    b0 = nc.dram_tensor("b0", [D], F32, kind="ExternalInput")
    W1 = nc.dram_tensor("W1", [D, D], F32, kind="ExternalInput")
    b1 = nc.dram_tensor("b1", [D], F32, kind="ExternalInput")
    Wh1 = nc.dram_tensor("Wh1", [CTX, H], F32, kind="ExternalInput")
    bh1 = nc.dram_tensor("bh1", [H], F32, kind="ExternalInput")
    Wh2s = nc.dram_tensor("Wh2s", [2, NGROUPS, 128, 2 * G * 256], FP8E3,
                          kind="ExternalInput")
    Wh2e = nc.dram_tensor("Wh2e", [H, 512], BF16, kind="ExternalInput")
    Bh2W1 = nc.dram_tensor("Bh2W1", [128, 2, M], BF16, kind="ExternalInput")
    Bh2W2 = nc.dram_tensor("Bh2W2", [128, 2, D], BF16, kind="ExternalInput")
    bh2r = nc.dram_tensor("bh2r", [2, 256], BF16, kind="ExternalInput")
    out = nc.dram_tensor("out", [BS, D], F32, kind="ExternalOutput")

    with tile.TileContext(nc) as tc:
        with (
            tc.tile_pool(name="consts", bufs=1) as consts,
            tc.tile_pool(name="wts", bufs=1) as wts,
            tc.tile_pool(name="acts", bufs=1) as acts,
            tc.tile_pool(name="accs", bufs=1) as accs,
            tc.tile_pool(name="cps", bufs=4) as cps,
            tc.tile_pool(name="tmps", bufs=4) as tmps,
            tc.tile_pool(name="wh2s", bufs=3) as wh2s,
            tc.tile_pool(name="pk", bufs=3, space="PSUM") as pkp,
            tc.tile_pool(name="pacc", bufs=1, space="PSUM") as paccp,
            tc.tile_pool(name="pmisc", bufs=1, space="PSUM") as pmisc,
        ):
            # ---- input DMAs ----
            xsb = acts.tile([128, 2, OBS], F32)
            nc.sync.dma_start(out=xsb[:], in_=x[:, :].rearrange("(hb p) o -> p hb o", p=128))
            ctxsb = acts.tile([128, 2, CTX], F32)
            nc.sync.dma_start(out=ctxsb[:], in_=ctx_in[:, :].rearrange("(hb p) o -> p hb o", p=128))
            W0sb = wts.tile([OBS, D], F32)
            nc.sync.dma_start(out=W0sb[:], in_=W0[:])
            W1sb = wts.tile([128, 2, D], F32)
            nc.sync.dma_start(out=W1sb[:], in_=W1[:, :].rearrange("(cc p) j -> p cc j", p=128))
            Wh1sb = wts.tile([CTX, H], F32)
            nc.sync.dma_start(out=Wh1sb[:], in_=Wh1[:])
            b0sb = wts.tile([128, 2], F32)
            nc.sync.dma_start(out=b0sb[:], in_=b0[:].rearrange("(cc p) -> p cc", p=128))
            b1sb = wts.tile([128, 2], F32)
            nc.sync.dma_start(out=b1sb[:], in_=b1[:].rearrange("(cc p) -> p cc", p=128))
            bh1sb = wts.tile([128, 2], F32)
            nc.sync.dma_start(out=bh1sb[:], in_=bh1[:].rearrange("(cc p) -> p cc", p=128))
            Bh2W1sb = wts.tile([128, 2, M], BF16)
            nc.sync.dma_start(out=Bh2W1sb[:], in_=Bh2W1[:])
            Bh2W2sb = wts.tile([128, 2, D], BF16)
            nc.sync.dma_start(out=Bh2W2sb[:], in_=Bh2W2[:])
            Wh2esb = wts.tile([128, 2, 512], BF16)
            nc.sync.dma_start(out=Wh2esb[:], in_=Wh2e[:, :].rearrange("(kc p) j -> p kc j", p=128))
            bh2rsb = wts.tile([1, 2, 256], BF16)
            nc.sync.dma_start(out=bh2rsb[:], in_=bh2r[None, :, :])

            ident = consts.tile([128, 128], F32)
            make_identity(nc, ident[:])
            identb = consts.tile([128, 128], BF16)
            make_identity(nc, identb[:])
            onesb = consts.tile([1, 128], BF16)
            nc.vector.memset(onesb[:], 1.0)
            zerow = consts.tile([128, 512], BF16)
            nc.vector.memset(zerow[:], 0.0)

            # ---- transposes of x / context ----
            xT = acts.tile([OBS, BS], F32)
            ctxT = acts.tile([CTX, BS], F32)
            for hb in range(2):
                pt = pmisc.tile([128, 256], F32, tag="pm")
                nc.tensor.transpose(pt[0:OBS, 0:128], xsb[:, hb, :], ident[:])
                nc.vector.tensor_copy(xT[:, hb * 128:(hb + 1) * 128], pt[0:OBS, 0:128])
                pt2 = pmisc.tile([128, 256], F32, tag="pm")
                nc.tensor.transpose(pt2[0:CTX, 0:128], ctxsb[:, hb, :], ident[:])
                nc.vector.tensor_copy(ctxT[:, hb * 128:(hb + 1) * 128], pt2[0:CTX, 0:128])

            # ---- main MLP ----
            h1T = acts.tile([128, 2, BS], F32)
            for dc in range(2):
                ph = pmisc.tile([128, 256], F32, tag="pm")
                nc.tensor.matmul(ph[:], W0sb[:, dc * 128:(dc + 1) * 128], xT[:],
                                 start=True, stop=True)
                nc.scalar.activation(h1T[:, dc, :], ph[:], AF.Relu, bias=b0sb[:, dc:dc + 1])
            hT = acts.tile([128, 2, BS], F32)
            for dc2 in range(2):
                ph = pmisc.tile([128, 256], F32, tag="pm")
                nc.tensor.matmul(ph[:], W1sb[:, 0, dc2 * 128:(dc2 + 1) * 128],
                                 h1T[:, 0, :], start=True, stop=False)
                nc.tensor.matmul(ph[:], W1sb[:, 1, dc2 * 128:(dc2 + 1) * 128],
                                 h1T[:, 1, :], start=False, stop=True)
                nc.scalar.activation(hT[:, dc2, :], ph[:], AF.Relu, bias=b1sb[:, dc2:dc2 + 1])
            hTb = acts.tile([128, 2, BS], BF16)
            nc.vector.tensor_copy(hTb[:], hT[:])

            # ---- hypernet first layer ----
            cT = acts.tile([128, 2, BS], F32)
            for cc in range(2):
                ph = pmisc.tile([128, 256], F32, tag="pm")
                nc.tensor.matmul(ph[:], Wh1sb[:, cc * 128:(cc + 1) * 128], ctxT[:],
                                 start=True, stop=True)
                nc.scalar.activation(cT[:, cc, :], ph[:], AF.Relu, bias=bh1sb[:, cc:cc + 1])
            cTb = acts.tile([128, 2, BS], BF16)
            nc.vector.tensor_copy(cTb[:], cT[:])

            # ---- b-major c (scaled 1/S) and h (skip) ----
            csb_s = acts.tile([128, 2, H], F32)
            for cc in range(2):
                for hb in range(2):
                    pt = pmisc.tile([128, 256], F32, tag="pm")
                    nc.tensor.transpose(pt[:, 0:128], cT[:, cc, hb * 128:(hb + 1) * 128], ident[:])
                    nc.vector.tensor_scalar(
                        csb_s[:, hb, cc * 128:(cc + 1) * 128], pt[:, 0:128],
                        1.0 / S, None, ALU.mult)
            csb_sb = acts.tile([128, 2, H], BF16)
            nc.vector.tensor_copy(csb_sb[:], csb_s[:])
            hsb = acts.tile([128, 2, D], F32)
            for dc in range(2):
                for hb in range(2):
                    pt = pmisc.tile([128, 256], F32, tag="pm")
                    nc.tensor.transpose(pt[:, 0:128], hT[:, dc, hb * 128:(hb + 1) * 128], ident[:])
                    nc.vector.tensor_copy(hsb[:, hb, dc * 128:(dc + 1) * 128], pt[:, 0:128])

            def stream_layer(layer, lhsTb):
                master = accs.tile([128, 2, 256], F32, tag=f"master{layer}")
                accD = accs.tile([128, 2, 2, 256], BF16, tag=f"accD{layer}")
                accG = accs.tile([128, 2, 256], BF16, tag=f"accG{layer}")
                firstD = [True, True]
                firstG = [True, True]
                kcntD = [0, 0]
                kcntG = [0, 0]
                BW = Bh2W1sb if layer == 0 else Bh2W2sb
                col = 0 if layer == 0 else 256

                pz = paccp.tile([128, 2, 256], F32, tag="pz")
                # one bank-wide group open (PSUM groups are bank-granular)
                nc.tensor.matmul(pz[:, :, :], identb[:], zerow[:],
                                 start=True, stop=False)
                for hb in range(2):
                    sl = slice(hb * 128, (hb + 1) * 128)
                    nc.tensor.matmul(pz[:, hb, :], lhsTb[:, 0, sl], BW[:, 0, :],
                                     start=False, stop=False)
                    nc.tensor.matmul(pz[:, hb, :], lhsTb[:, 1, sl], BW[:, 1, :],
                                     start=False, stop=False)
                    nc.tensor.matmul(pz[:, hb, :], cTb[:, 0, sl],
                                     Wh2esb[:, 0, col:col + 256], start=False, stop=False)
                    nc.tensor.matmul(pz[:, hb, :], cTb[:, 1, sl],
                                     Wh2esb[:, 1, col:col + 256], start=False, stop=False)
                    nc.tensor.matmul(pz[:, hb, :], onesb[:], bh2rsb[:, layer, :],
                                     start=False, stop=False)
                    if layer == 1:
                        nc.tensor.matmul(pz[:, hb, :], ident[:], hsb[:, hb, :],
                                         start=False, stop=False)

                def flushD(hb):
                    if firstD[hb]:
                        nc.vector.tensor_copy(master[:, hb, :], accD[:, hb, 0, :])
                        firstD[hb] = False
                    else:
                        nc.vector.tensor_tensor(master[:, hb, :], master[:, hb, :],
                                                accD[:, hb, 0, :], ALU.add)
                    nc.vector.tensor_tensor(master[:, hb, :], master[:, hb, :],
                                            accD[:, hb, 1, :], ALU.add)

                def flushG(hb):
                    nc.vector.tensor_tensor(master[:, hb, :], master[:, hb, :],
                                            accG[:, hb, :], ALU.add)

                si = [0, 0]
                for g in range(NGROUPS):
                    wt = wh2s.tile([128, 2, G, 256], FP8E3, tag="wt")
                    nc.sync.dma_start(
                        out=wt[:], in_=Wh2s[layer, g, :, :].rearrange(
                            "p (dc kk m) -> p dc kk m", dc=2, m=256))
                    for hb in range(2):
                        sl = slice(hb * 128, (hb + 1) * 128)
                        for sp in range(NSLABS // 2):
                          pks = []
                          for j in range(2):
                              pkt = pkp.tile([128, SLAB, 256], F32, tag="pk")
                              pks.append(pkt)
                          # dc-major fill: lhsT runs of 4 keep LDWEIGHTS off
                          # the critical path
                          for dc in range(2):
                              for j in range(2):
                                  for pp in range(SLAB // 2):
                                      sg = sp * 2 + j
                                      ks = sg * SLAB + 2 * pp
                                      nc.tensor.matmul(
                                          pks[j][:, 2 * pp:2 * pp + 2, :],
                                          lhsTb[:, dc, sl],
                                          wt[:, dc, ks:ks + 2, :],
                                          start=(dc == 0), stop=(dc == 1))
                          for j in range(2):
                            sg = sp * 2 + j
                            if True:
                                k0 = g * G + sg * SLAB
                                pk = pks[j]
                                lane = SCHEDULE[si[hb]]
                                si[hb] += 1
                            if lane == 'G':
                                cp = cps.tile([128, SLAB, 256], BF16, tag="cp")
                                nc.scalar.activation(
                                    cp[:].rearrange("p a m -> p (a m)"),
                                    pk[:].rearrange("p a m -> p (a m)"), AF.Copy)
                                # gpsimd: slab-wide broadcast multiply + tree add
                                tg = tmps.tile([128, SLAB, 256], BF16, tag="tg")
                                nc.gpsimd.tensor_tensor(
                                    tg[:], cp[:],
                                    csb_sb[:, hb, k0:k0 + SLAB, None]
                                    .broadcast_to([128, SLAB, 256]), ALU.mult)
                                nc.gpsimd.tensor_tensor(
                                    tg[:, 0:2, :], tg[:, 0:2, :], tg[:, 2:4, :],
                                    ALU.add)
                                if firstG[hb]:
                                    nc.gpsimd.tensor_tensor(
                                        accG[:, hb, :], tg[:, 0, :], tg[:, 1, :],
                                        ALU.add)
                                    firstG[hb] = False
                                else:
                                    nc.gpsimd.tensor_tensor(
                                        accG[:, hb, :], accG[:, hb, :], tg[:, 0, :],
                                        ALU.add)
                                    nc.gpsimd.tensor_tensor(
                                        accG[:, hb, :], accG[:, hb, :], tg[:, 1, :],
                                        ALU.add)
                                kcntG[hb] += SLAB
                                if kcntG[hb] >= CHUNK:
                                    kcntG[hb] = 0
                                    firstG[hb] = True
                                    flushG(hb)
                            elif lane == 'A':
                                for kk in range(SLAB):
                                    k = k0 + kk
                                    par = kk % 2
                                    if kcntD[hb] < 2:
                                        nc.vector.tensor_scalar(
                                            accD[:, hb, par, :], pk[:, kk, :],
                                            csb_s[:, hb, k:k + 1], None, ALU.mult)
                                    else:
                                        nc.vector.scalar_tensor_tensor(
                                            accD[:, hb, par, :], pk[:, kk, :],
                                            csb_s[:, hb, k:k + 1], accD[:, hb, par, :],
                                            op0=ALU.mult, op1=ALU.add)
                                    kcntD[hb] += 1
                                    if kcntD[hb] == CHUNK:
                                        kcntD[hb] = 0
                                        flushD(hb)
                            else:  # 'P'
                                tmp = tmps.tile([128, SLAB, 256], BF16, tag="tmp")
                                for kk in range(SLAB):
                                    k = k0 + kk
                                    nc.scalar.activation(
                                        tmp[:, kk, :], pk[:, kk, :], AF.Copy,
                                        scale=csb_s[:, hb, k:k + 1])
                                for kk in range(SLAB):
                                    nc.tensor.matmul(pz[:, hb, :], identb[:],
                                                     tmp[:, kk, :],
                                                     start=False, stop=False)

                # drain + close
                for hb in range(2):
                    if kcntD[hb] != 0:
                        kcntD[hb] = 0
                        flushD(hb)
                    if kcntG[hb] != 0:
                        kcntG[hb] = 0
                        flushG(hb)


                nc.tensor.matmul(pz[:, :, :], identb[:], zerow[:],
                                 start=False, stop=True)
                zpre = acts.tile([128, 2, 256], F32, tag=f"zpre{layer}")
                for hb in range(2):
                    nc.vector.tensor_tensor(zpre[:, hb, :], master[:, hb, :],
                                            pz[:, hb, :], ALU.add)
                return zpre

            # ---- layer 1 ----
            z1 = stream_layer(0, hTb)
            zrel = acts.tile([128, 2, M], F32)
            for hb in range(2):
                nc.scalar.activation(zrel[:, hb, :], z1[:, hb, :], AF.Relu)
            zTb = acts.tile([128, 2, BS], BF16)
            for mc in range(2):
                for hb in range(2):
                    pt = pmisc.tile([128, 256], F32, tag="pm")
                    nc.tensor.transpose(pt[:, 0:128],
                                        zrel[:, hb, mc * 128:(mc + 1) * 128], ident[:])
                    nc.vector.tensor_copy(zTb[:, mc, hb * 128:(hb + 1) * 128], pt[:, 0:128])

            # ---- layer 2 ----
            z2 = stream_layer(1, zTb)
            orel = acts.tile([128, 2, D], F32)
            for hb in range(2):
                nc.scalar.activation(orel[:, hb, :], z2[:, hb, :], AF.Relu)
            nc.sync.dma_start(
                out=out[:, :].rearrange("(hb p) d -> p hb d", p=128), in_=orel[:])


    nc.compile()
    return nc


def _stage(inputs):
    f = {k: np.ascontiguousarray(np.asarray(v, dtype=np.float32)) for k, v in inputs.items()}
    Wh2 = f.pop("Wh2")
    bh2 = f.pop("bh2")
    bf = ml_dtypes.bfloat16
    e3 = ml_dtypes.float8_e3m4

    W1p = Wh2[:, :I0].reshape(H, D, M)
    W2p = Wh2[:, I1:I2].reshape(H, M, D)
    st = np.stack([W1p, W2p])                             # [l, k, c, o]
    st = st.reshape(2, NGROUPS, G, 2, 128, 256)           # l, g, kk, dc, p, m
    st = st.transpose(0, 1, 4, 3, 2, 5)                   # l, g, p, dc, kk, m
    Wh2s = (st * S).astype(e3).reshape(2, NGROUPS, 128, 2 * G * 256)

    Wh2e = np.concatenate([Wh2[:, I0:I1], Wh2[:, I2:]], axis=1).astype(bf)
    Bh2W1 = bh2[:I0].reshape(2, 128, M).transpose(1, 0, 2).astype(bf)
    Bh2W2 = bh2[I1:I2].reshape(2, 128, D).transpose(1, 0, 2).astype(bf)
    bh2r = np.stack([bh2[I0:I1], bh2[I2:]]).astype(bf)

    f["Wh2s"] = np.ascontiguousarray(Wh2s)
    f["Wh2e"] = np.ascontiguousarray(Wh2e)
    f["Bh2W1"] = np.ascontiguousarray(Bh2W1)
    f["Bh2W2"] = np.ascontiguousarray(Bh2W2)
    f["bh2r"] = np.ascontiguousarray(bh2r)
    return f


def _in_maps(inputs):
    full = _stage(inputs)
    maps = []
    for i in range(NCORES):
        m = dict(full)
        m["x"] = full["x"][i * BS:(i + 1) * BS]
        m["context"] = full["context"][i * BS:(i + 1) * BS]
        maps.append(m)
    return maps


def _get_nc():
    global _CACHED_NC
    if _CACHED_NC is None:
        _CACHED_NC = build_nc()
    return _CACHED_NC


def run_spmd(inputs, trace=False):
    from concourse.bass_utils import run_bass_kernel_spmd

    nc = _get_nc()
    res = run_bass_kernel_spmd(nc, _in_maps(inputs), list(range(NCORES)), trace=trace)
    out = np.concatenate([res.results[i]["out"] for i in range(NCORES)], axis=0)
    return out, res


def kernel(**inputs) -> np.ndarray:
    out, _ = run_spmd(inputs, trace=False)
    return out

